# revision 1
# baseline (speedup 1.0000x reference)
"""RNN-T JointNetwork kernel for 8x Trainium2 NeuronCores.

Sharding: data-parallel over batch (B=8 -> 1 batch element per core).
Each core computes its (T, U, V) logit block fully on-chip:
  enc proj (200,512)@(512,640), pred proj (50,640)@(640,640),
  joint = tanh(enc[:,None,:] + pred[None,:,:] + b), out = joint @ W_out.T.
b_out is added on the host during the gather (saves a per-element DVE op
on the critical path).

Host-side prep transposes + casts operands to bf16 so the device kernel
needs no on-chip transposes: all matmuls contract along the SBUF
partition dim. The output is produced in [V, T*U] (transposed) layout so
PSUM tiles [128 v, 500 p] DMA out contiguously; the host transposes back.
"""

import numpy as np
import ml_dtypes

P = 128
B, T, U = 8, 200, 50
DE, DP, DJ, V = 512, 640, 640, 1024
NDE, NDP, NJC, NVC = DE // P, DP // P, DJ // P, V // P  # 4, 5, 5, 8
TB = 10              # t-block per inner iteration
PBLK = TB * U        # 500 joint positions per block (one PSUM bank)
NPB = T // TB        # 20 blocks
VQ = 4               # v-chunks ganged per PSUM tile (4 banks)
NVH = NVC // VQ      # 2 v-halves

BF16 = ml_dtypes.bfloat16

_module = None


def _build_module():
    import concourse.bass as bass
    import concourse.mybir as mybir
    import concourse.tile as tile
    from concourse import bacc

    bf = mybir.dt.bfloat16
    f32 = mybir.dt.float32
    Alu = mybir.AluOpType
    Act = mybir.ActivationFunctionType
    ts, ds = bass.ts, bass.ds

    nc = bacc.Bacc("TRN2", target_bir_lowering=False, debug=False)

    d_encT = nc.dram_tensor("encT", (P, NDE, T), bf, kind="ExternalInput").ap()
    d_predT = nc.dram_tensor("predT", (P, NDP, U), bf, kind="ExternalInput").ap()
    d_wencT = nc.dram_tensor("wencT", (P, NDE, DJ), bf, kind="ExternalInput").ap()
    d_wpredT = nc.dram_tensor("wpredT", (P, NDP, DJ), bf, kind="ExternalInput").ap()
    d_woutT = nc.dram_tensor("woutT", (P, NJC, V), bf, kind="ExternalInput").ap()
    d_bj = nc.dram_tensor("bj", (P, NJC), f32, kind="ExternalInput").ap()
    d_out = nc.dram_tensor("out", (V, T * U), bf, kind="ExternalOutput").ap()

    with tile.TileContext(nc) as tc:
        with (
            tc.tile_pool(name="consts", bufs=1) as consts,
            tc.tile_pool(name="sums", bufs=6) as sums,
            tc.tile_pool(name="joints", bufs=15) as joints,
            tc.tile_pool(name="outsb", bufs=6) as outsb,
            tc.tile_pool(name="ps", bufs=8, space="PSUM") as pspool,
        ):
            # enc path first: the first projection matmuls only need
            # wenc+encT, so don't queue 4MB of other weights ahead of them.
            wenc = consts.tile([P, NDE, DJ], bf)
            nc.sync.dma_start(wenc[:], d_wencT[:])
            encT = consts.tile([P, NDE, T], bf)
            nc.sync.dma_start(encT[:], d_encT[:])
            predT = consts.tile([P, NDP, U], bf)
            nc.sync.dma_start(predT[:], d_predT[:])
            bj = consts.tile([P, NJC], f32)
            nc.sync.dma_start(bj[:], d_bj[:])
            wpred = consts.tile([P, NDP, DJ], bf)
            nc.sync.dma_start(wpred[:], d_wpredT[:])
            wout = consts.tile([P, NJC, V], bf)
            nc.sync.dma_start(wout[:], d_woutT[:])

            # --- projections -> encP[j,t] (bf16), predU[j, t-rep, u] (bf16,
            # (b_enc+b_pred) folded in)
            encP = consts.tile([P, NJC, T], f32)
            predU = consts.tile([P, NJC, TB, U], f32)
            for jc in range(NJC):
                ps_e = pspool.tile([P, 512], f32, tag="ps")
                for dc in range(NDE):
                    nc.tensor.matmul(
                        ps_e[:, :T], wenc[:, dc, ts(jc, P)], encT[:, dc, :],
                        start=(dc == 0), stop=(dc == NDE - 1),
                    )
                nc.vector.tensor_copy(encP[:, jc, :], ps_e[:, :T])

                ps_p = pspool.tile([P, 512], f32, tag="ps")
                for dc in range(NDP):
                    nc.tensor.matmul(
                        ps_p[:, :U], wpred[:, dc, ts(jc, P)], predT[:, dc, :],
                        start=(dc == 0), stop=(dc == NDP - 1),
                    )
                # biased pred row, replicated TB times so the joint add is
                # a plain elementwise op against a last-dim-broadcast enc.
                nc.vector.tensor_tensor(
                    predU[:, jc, 0, :], ps_p[:, :U],
                    bj[:, jc, None].to_broadcast((P, U)), Alu.add,
                )
                for r in range(1, TB):
                    nc.vector.tensor_copy(predU[:, jc, r, :], predU[:, jc, 0, :])

            # --- main loop over t-blocks
            for pb in range(NPB):
                jtiles = []
                for jc in range(NJC):
                    sum_t = sums.tile([P, TB, U], f32, tag="sum")
                    nc.vector.tensor_tensor(
                        sum_t[:],
                        encP[:, jc, ts(pb, TB), None].to_broadcast((P, TB, U)),
                        predU[:, jc],
                        Alu.add,
                    )
                    jt = joints.tile([P, TB, U], bf, tag="jt")
                    nc.scalar.activation(jt[:], sum_t[:], Act.Tanh)
                    jtiles.append(jt[:].rearrange("p a b -> p (a b)"))

                for vh in range(NVH):
                    osb = outsb.tile([P, VQ, PBLK], bf, tag="osb")
                    for vq in range(VQ):
                        ps_o = pspool.tile([P, 512], f32, tag="ps")
                        for jc in range(NJC):
                            nc.tensor.matmul(
                                ps_o[:, :PBLK], wout[:, jc, ts(vh * VQ + vq, P)],
                                jtiles[jc],
                                start=(jc == 0), stop=(jc == NJC - 1),
                            )
                        nc.vector.tensor_copy(osb[:, vq, :], ps_o[:, :PBLK])
                    nc.sync.dma_start(
                        d_out[ds(vh * VQ * P, VQ * P), ts(pb, PBLK)]
                        .rearrange("(q p) c -> p q c", p=P),
                        osb[:],
                    )

    nc.compile()
    return nc


def _get_module():
    global _module
    if _module is None:
        _module = _build_module()
    return _module


def _chunk(x2d, dtype=BF16):
    """(n*128, C...) -> (128, n, C...) partition-chunked, contiguous."""
    n = x2d.shape[0] // P
    return np.ascontiguousarray(
        x2d.reshape((n, P) + x2d.shape[1:]).swapaxes(0, 1)
    ).astype(dtype)


def make_in_maps(encoder_out, predictor_out, W_enc, b_enc, W_pred, b_pred, W_out, b_out):
    wencT = _chunk(np.ascontiguousarray(W_enc.T))       # (128, 4, 640)
    wpredT = _chunk(np.ascontiguousarray(W_pred.T))     # (128, 5, 640)
    woutT = _chunk(np.ascontiguousarray(W_out.T))       # (128, 5, 1024)
    bj = np.ascontiguousarray(
        (b_enc + b_pred).reshape(NJC, P).T).astype(np.float32)   # (128, 5)
    in_maps = []
    for b in range(B):
        in_maps.append({
            "encT": _chunk(np.ascontiguousarray(encoder_out[b].T)),    # (128,4,200)
            "predT": _chunk(np.ascontiguousarray(predictor_out[b].T)), # (128,5,50)
            "wencT": wencT,
            "wpredT": wpredT,
            "woutT": woutT,
            "bj": bj,
        })
    return in_maps


def _postprocess(out_vt, b_out):
    """(V, T*U) device output (bf16) -> (T, U, V) fp32 with vocab bias."""
    return out_vt.astype(np.float32).T.reshape(T, U, V) + b_out.astype(np.float32)


def kernel(encoder_out, predictor_out, W_enc, b_enc, W_pred, b_pred, W_out, b_out):
    from concourse.bass_utils import run_bass_kernel_spmd

    nc = _get_module()
    in_maps = make_in_maps(
        encoder_out, predictor_out, W_enc, b_enc, W_pred, b_pred, W_out, b_out
    )
    res = run_bass_kernel_spmd(nc, in_maps, list(range(B)))
    out = np.empty((B, T, U, V), np.float32)
    for b in range(B):
        out[b] = _postprocess(res.results[b]["out"], b_out)
    return out



# revision 2
# speedup vs baseline: 1.2508x; 1.2508x over previous
"""RNN-T JointNetwork kernel for 8x Trainium2 NeuronCores.

Sharding: data-parallel over batch (B=8 -> 1 batch element per core).

Layout/pipeline summary:
  enc proj  -> encP [128, NJC, T] bf16
  pred proj -> predR [128, NJC, U, TB] bf16 (bias folded, replicated over t)
  joint     jt[j, u, t] = tanh(encP[j,t] + predR[j,u,t])  (DVE 2x-mode add)
  out proj  psum[v, u*TB+t] += woutV[j,v] @ jt   (500-col matmuls)

v4 structure:
- Inputs arrive in few, large DMAs ordered by consumption: one "hot pack"
  (encT | predT | bias | jc0 weights | vq0 out-weights), then per-jc merged
  enc+pred weight chunks, then per-vq out-weight chunks.  (Descriptor-gen
  and the DMA engines are serial resources; count and order are what matter.)
- The first vocab chunk's matmul rows are interleaved into the projection
  loop, so the PE works through DMA waits.  Projections accumulate in a
  dedicated 2-bank PSUM pool; the main loop ping-pongs G=3-bank vq-groups
  on the remaining 6 banks.
- Matmuls are jc-outer / t-block-inner: 3 consecutive matmuls share one
  stationary weight tile.
- PSUM drains split DVE (vq 0-4) / ACT (vq 5-7); next group's add/tanh
  emission is interleaved so neither FIFO dams the PE at group boundaries.
- The kernel's final vocab chunk runs t-block-outer with parity-split
  drains and per-block DMAs to minimize the post-matmul tail.

b_out is added on the host during the gather.  Device output is
[V, NPB, U, TB]-ordered; the host restores (T, U, V).
"""

import numpy as np
import ml_dtypes

P = 128
B, T, U = 8, 200, 50
DE, DP, DJ, V = 512, 640, 640, 1024
NDE, NDP, NJC, NVC = DE // P, DP // P, DJ // P, V // P  # 4, 5, 5, 8
TB = 10              # t per block
PBLK = U * TB        # 500 positions per block (one PSUM bank)
NPB = T // TB        # 20 blocks
G = 3                # t-blocks per matmul group (half the main PSUM ring)
GROUPS = [list(range(s, min(s + G, NPB))) for s in range(0, NPB, G)]  # 6x3 + 1x2
NG = len(GROUPS)

# hot-pack segment offsets (bf16 elements per partition)
# hotA: acts + jc0 weights (everything the first projection stage needs);
# hotB: vq0+vq1 out-weight chunks.
SEG_ENCT = 0
SEG_PREDT = SEG_ENCT + NDE * T
SEG_BJ = SEG_PREDT + NDP * U
SEG_WENC0 = SEG_BJ + NJC
SEG_WPRED0 = SEG_WENC0 + NDE * P
HOTA = SEG_WPRED0 + NDP * P
HOTB = 2 * NJC * P

BF16 = ml_dtypes.bfloat16

_module = None


def _build_module():
    import concourse.bass as bass
    import concourse.mybir as mybir
    import concourse.tile as tile
    from concourse import bacc

    bf = mybir.dt.bfloat16
    f32 = mybir.dt.float32
    Alu = mybir.AluOpType
    Act = mybir.ActivationFunctionType
    ts, ds = bass.ts, bass.ds

    nc = bacc.Bacc("TRN2", target_bir_lowering=False, debug=False)

    d_hota = nc.dram_tensor("hota", (P, HOTA), bf, kind="ExternalInput").ap()
    d_hotb = nc.dram_tensor("hotb", (P, HOTB), bf, kind="ExternalInput").ap()
    d_wj = nc.dram_tensor("wj", (P, NJC - 1, (NDE + NDP) * P), bf,
                          kind="ExternalInput").ap()
    d_wv = nc.dram_tensor("wv", (P, NVC - 2, NJC * P), bf,
                          kind="ExternalInput").ap()
    d_out = nc.dram_tensor("out", (V, T * U), bf, kind="ExternalOutput").ap()

    with tile.TileContext(nc) as tc:
        with (
            tc.tile_pool(name="consts", bufs=1) as consts,
            tc.tile_pool(name="sums", bufs=10) as sums,
            tc.tile_pool(name="joints", bufs=2 * NJC * G) as joints,
            tc.tile_pool(name="outsb", bufs=8) as outsb,
            tc.tile_pool(name="psj", bufs=2, space="PSUM") as psj,
            tc.tile_pool(name="psm", bufs=2 * G, space="PSUM") as psm,
        ):
            # --- input DMAs (order == consumption order)
            hota = consts.tile([P, HOTA], bf)
            nc.sync.dma_start(hota[:], d_hota[:])
            hotb = consts.tile([P, HOTB], bf)
            nc.scalar.dma_start(hotb[:], d_hotb[:])
            wj = consts.tile([P, NJC - 1, (NDE + NDP) * P], bf)
            for j in range(NJC - 1):
                qe = nc.scalar if j % 2 == 0 else nc.sync
                qe.dma_start(wj[:, j], d_wj[:, j])
            wv = consts.tile([P, NVC - 2, NJC * P], bf)
            for v in range(NVC - 2):
                qe = nc.scalar if v % 2 == 0 else nc.sync
                qe.dma_start(wv[:, v], d_wv[:, v])

            encT = hota[:, SEG_ENCT:SEG_PREDT].rearrange("p (d t) -> p d t", d=NDE)
            predT = hota[:, SEG_PREDT:SEG_BJ].rearrange("p (d u) -> p d u", d=NDP)
            bjb = hota[:, SEG_BJ:SEG_WENC0]

            def wencv(jc, dc):
                if jc == 0:
                    return hota[:, SEG_WENC0 + dc * P: SEG_WENC0 + (dc + 1) * P]
                return wj[:, jc - 1, dc * P:(dc + 1) * P]

            def wpredv(jc, dc):
                if jc == 0:
                    return hota[:, SEG_WPRED0 + dc * P: SEG_WPRED0 + (dc + 1) * P]
                return wj[:, jc - 1, (NDE + dc) * P:(NDE + dc + 1) * P]

            def woutv(vq, jc):
                if vq < 2:
                    return hotb[:, vq * NJC * P + jc * P: vq * NJC * P + (jc + 1) * P]
                return wv[:, vq - 2, jc * P:(jc + 1) * P]

            encP = consts.tile([P, NJC, T], bf)
            predR = consts.tile([P, NJC, U, TB], bf)
            pred0 = consts.tile([P, NJC, U], bf)
            bjf = consts.tile([P, NJC], f32)
            nc.vector.tensor_copy(bjf[:], bjb)

            def make_jt(pb, jc, enc_src=None):
                sm = sums.tile([P, U, TB], bf, tag="sum", name=f"sm_{pb}_{jc}")
                src_ap = (
                    enc_src[:, None, ts(pb, TB)].to_broadcast((P, U, TB))
                    if enc_src is not None
                    else encP[:, jc, None, ts(pb, TB)].to_broadcast((P, U, TB))
                )
                nc.vector.tensor_tensor(sm[:], src_ap, predR[:, jc], Alu.add)
                jt = joints.tile([P, U, TB], bf, tag="jt", name=f"jt_{pb}_{jc}")
                nc.scalar.activation(jt[:], sm[:], Act.Tanh)
                return jt[:].rearrange("p a b -> p (a b)")

            jts = {}

            # --- projections, with group 0's joint tiles and the first two
            # vocab chunks' matmul rows interleaved per jc (lagged one stage
            # so rows never wait on the add/tanh chain).
            pss0 = [
                psm.tile([P, 512], f32, tag="ps", name=f"ps_p0_{s}")
                for s in range(G)
            ]
            pss1 = [
                psm.tile([P, 512], f32, tag="ps", name=f"ps_p1_{s}")
                for s in range(G)
            ]

            def vrows(jc):
                for vq, pst in ((0, pss0), (1, pss1)):
                    for s in range(G):
                        nc.tensor.matmul(
                            pst[s][:, :PBLK], woutv(vq, jc), jts[(s, jc)],
                            start=(jc == 0), stop=(jc == NJC - 1),
                        )

            for jc in range(NJC):
                ps_e = psj.tile([P, 512], f32, tag="psj", name=f"pse_{jc}")
                for dc in range(NDE):
                    nc.tensor.matmul(
                        ps_e[:, :T], wencv(jc, dc), encT[:, dc],
                        start=(dc == 0), stop=(dc == NDE - 1),
                    )
                nc.vector.tensor_copy(encP[:, jc], ps_e[:, :T])

                ps_p = psj.tile([P, 512], f32, tag="psj", name=f"psp_{jc}")
                for dc in range(NDP):
                    nc.tensor.matmul(
                        ps_p[:, :U], wpredv(jc, dc), predT[:, dc],
                        start=(dc == 0), stop=(dc == NDP - 1),
                    )
                nc.vector.tensor_scalar_add(pred0[:, jc], ps_p[:, :U], bjf[:, jc, None])
                nc.vector.tensor_copy(
                    predR[:, jc], pred0[:, jc, :, None].to_broadcast((P, U, TB))
                )
                for s in GROUPS[0]:
                    jts[(s, jc)] = make_jt(
                        s, jc, enc_src=ps_e if jc == NJC - 1 else None
                    )
                if jc > 0:
                    vrows(jc - 1)
            vrows(NJC - 1)

            def drain(osb, k, ps_t, engine):
                if engine == "v":
                    nc.vector.tensor_copy(osb[:, k], ps_t[:, :PBLK])
                else:
                    nc.scalar.copy(osb[:, k], ps_t[:, :PBLK])

            for vq, pst in ((0, pss0), (1, pss1)):
                osb0 = outsb.tile([P, G, PBLK], bf, tag="osb", name=f"osb_p{vq}")
                for s in range(G):
                    drain(osb0, s, pst[s], "v" if (s + vq) % 2 == 0 else "a")
                nc.sync.dma_start(
                    d_out[ds(vq * P, P), ds(0, G * PBLK)],
                    osb0[:].rearrange("p a b -> p (a b)"),
                )

            # --- main loop
            for g in range(NG):
                pbs = GROUPS[g]
                gl = len(pbs)
                last_g = g == NG - 1
                nxt = (
                    [(pb, jc) for jc in range(NJC) for pb in GROUPS[g + 1]]
                    if not last_g else []
                )
                ni = 0
                vqs = list(range(2, NVC)) if g == 0 else list(range(NVC))
                for vi, vq in enumerate(vqs):
                    last_vq = last_g and vi == len(vqs) - 1
                    pss = [
                        psm.tile([P, 512], f32, tag="ps", name=f"ps_{g}_{vq}_{k}")
                        for k in range(gl)
                    ]
                    osb = outsb.tile([P, gl, PBLK], bf, tag="osb",
                                     name=f"osb_{g}_{vq}")
                    if not last_vq:
                        # jc-outer: gl consecutive matmuls share one weight.
                        for jc in range(NJC):
                            for k, pb in enumerate(pbs):
                                nc.tensor.matmul(
                                    pss[k][:, :PBLK], woutv(vq, jc), jts[(pb, jc)],
                                    start=(jc == 0), stop=(jc == NJC - 1),
                                )
                        for k in range(gl):
                            drain(osb, k, pss[k], "v" if vq < 6 else "a")
                        nc.sync.dma_start(
                            d_out[ds(vq * P, P), ds(pbs[0] * PBLK, gl * PBLK)],
                            osb[:].rearrange("p a b -> p (a b)"),
                        )
                    else:
                        # final tile: pb-outer so each bank finishes, drains,
                        # and ships as early as possible.
                        for k, pb in enumerate(pbs):
                            for jc in range(NJC):
                                nc.tensor.matmul(
                                    pss[k][:, :PBLK], woutv(vq, jc), jts[(pb, jc)],
                                    start=(jc == 0), stop=(jc == NJC - 1),
                                )
                            drain(osb, k, pss[k], "v" if k % 2 == 0 else "a")
                            nc.sync.dma_start(
                                d_out[ds(vq * P, P), ts(pb, PBLK)], osb[:, k],
                            )
                    quota = 4 if vi < 2 else (3 if vi < 3 else 2)
                    for _ in range(quota):
                        if ni < len(nxt):
                            pb, jc = nxt[ni]
                            jts[(pb, jc)] = make_jt(pb, jc)
                            ni += 1
                while ni < len(nxt):
                    pb, jc = nxt[ni]
                    jts[(pb, jc)] = make_jt(pb, jc)
                    ni += 1

    # Drop consecutive duplicate Ldweights: the jc-outer/t-block-inner matmul
    # order makes runs of matmuls share one stationary weight tile, but the
    # lowering emits an Ldweights per matmul.  The PE keeps the loaded weights
    # until the next Ldweights, so repeats (which carry no waits/updates at
    # this stage -- guarded below) are pure overhead on the PE queue.
    for blk in nc.m.functions[0].blocks:
        prev_key = None
        keep = []
        for inst in blk.instructions:
            if inst.opcode == "Ldweights":
                key = (str(inst.ins[0]), str(inst.perf_mode), str(inst.is_transpose),
                       str(inst.tile_position), str(inst.tile_size))
                si = inst.sync_info
                clean = si is None or (len(si.on_wait) == 0 and len(si.on_update) == 0)
                if key == prev_key and clean:
                    continue
                prev_key = key
            elif inst.opcode != "Matmult":
                pass  # non-PE instructions cannot clobber PE weights
            keep.append(inst)
        if len(keep) != len(blk.instructions):
            blk.instructions[:] = keep

    nc.compile()
    return nc


def _get_module():
    global _module
    if _module is None:
        _module = _build_module()
    return _module


def _chunk(x2d, dtype=BF16):
    """(n*128, C...) -> (128, n, C...) partition-chunked, contiguous."""
    n = x2d.shape[0] // P
    return np.ascontiguousarray(
        x2d.reshape((n, P) + x2d.shape[1:]).swapaxes(0, 1)
    ).astype(dtype)


def make_in_maps(encoder_out, predictor_out, W_enc, b_enc, W_pred, b_pred, W_out, b_out):
    # per-jc weight chunks: [P(part of d), jc, out-cols]
    wencJ = np.ascontiguousarray(
        W_enc.T.reshape(NDE, P, NJC, P).transpose(1, 2, 0, 3)).astype(BF16)
    wpredJ = np.ascontiguousarray(
        W_pred.T.reshape(NDP, P, NJC, P).transpose(1, 2, 0, 3)).astype(BF16)
    woutV = np.ascontiguousarray(
        W_out.T.reshape(NJC, P, NVC, P).transpose(1, 2, 0, 3)).astype(BF16)
    bjb = np.ascontiguousarray(
        (b_enc + b_pred).reshape(NJC, P).T).astype(BF16)        # (128, NJC)

    wj = np.concatenate(
        [
            np.concatenate(
                [wencJ[:, jc].reshape(P, -1), wpredJ[:, jc].reshape(P, -1)], axis=1
            )[:, None]
            for jc in range(1, NJC)
        ],
        axis=1,
    )  # (P, NJC-1, (NDE+NDP)*P)
    wv = np.ascontiguousarray(
        woutV[:, 2:].reshape(P, NVC - 2, NJC * P))

    in_maps = []
    for b in range(B):
        encTb = _chunk(np.ascontiguousarray(encoder_out[b].T))     # (128,4,200)
        predTb = _chunk(np.ascontiguousarray(predictor_out[b].T))  # (128,5,50)
        hota = np.concatenate(
            [
                encTb.reshape(P, -1), predTb.reshape(P, -1), bjb,
                wencJ[:, 0].reshape(P, -1), wpredJ[:, 0].reshape(P, -1),
            ],
            axis=1,
        ).astype(BF16)
        hotb = np.concatenate(
            [woutV[:, 0].reshape(P, -1), woutV[:, 1].reshape(P, -1)], axis=1
        ).astype(BF16)
        assert hota.shape == (P, HOTA) and hotb.shape == (P, HOTB)
        in_maps.append({
            "hota": np.ascontiguousarray(hota),
            "hotb": np.ascontiguousarray(hotb),
            "wj": np.ascontiguousarray(wj),
            "wv": wv,
        })
    return in_maps


def _postprocess(out_vt, b_out):
    """(V, T*U) device output (bf16, [V, NPB, U, TB] order) -> (T, U, V) fp32."""
    arr = out_vt.astype(np.float32).reshape(V, NPB, U, TB)
    return arr.transpose(1, 3, 2, 0).reshape(T, U, V) + b_out.astype(np.float32)


def kernel(encoder_out, predictor_out, W_enc, b_enc, W_pred, b_pred, W_out, b_out):
    from concourse.bass_utils import run_bass_kernel_spmd

    nc = _get_module()
    in_maps = make_in_maps(
        encoder_out, predictor_out, W_enc, b_enc, W_pred, b_pred, W_out, b_out
    )
    res = run_bass_kernel_spmd(nc, in_maps, list(range(B)))
    out = np.empty((B, T, U, V), np.float32)
    for b in range(B):
        out[b] = _postprocess(res.results[b]["out"], b_out)
    return out
